# revision 7
# baseline (speedup 1.0000x reference)
"""Trainium2 Bass kernel for nn_DecoderRNN (LSTM decoder + vocab projection), v2.

Strategy (8 NeuronCores): vocab-shard the output projection 8-way; replicate
the LSTM recurrence on every core.

Layout: gates live in PSUM as [gate-dim (partitions), token (free)]:
  - h-part matmuls stream hT with N=64 (bf16): 4096 cyc/step, FLOP-optimal.
  - h is produced by the DVE directly in [H, token] layout -> no per-step PE
    transposes, no odd-step re-injection.
  - x_proj runs in fp8e4m3 DoubleRow (2 K-chunks/pass at 0.5 cyc/row = 4x
    bf16 throughput). Operands pre-scaled x4/x4 host-side (avoids fp8
    subnormals); W_hh pre-scaled x16 so the PSUM gates are 16x true scale,
    descaled for free via activation(scale=1/16).
  - the t=0 gates (image features, unit-scale: too coarse for fp8) are
    computed on the HOST and read by step 0's activations from SBUF.
  - vocab projection is bf16, unpadded (512/512/226 groups), drained to an
    fp16 staging tile (output DMA'd as fp16, host converts to f32).
"""

import numpy as np
import ml_dtypes

import concourse.bacc as bacc
import concourse.mybir as mybir
import concourse.tile as tile
from concourse.bass import IndirectOffsetOnAxis
from concourse.bass_utils import run_bass_kernel_spmd
from concourse.masks import make_identity

B, T, E, H, V = 64, 32, 512, 512, 10000
G4 = 4 * H            # 2048 gate dims
NTOK = B * T          # 2048 tokens (token-major: tok = t*64 + b)
NCORES = 8
VL = V // NCORES      # 1250 vocab per core
KE = E // 128         # 4 K-chunks over E
KH = H // 128         # 4 K-chunks over H
NCHUNK = NTOK // 128  # 16 token chunks (= pairs of steps)

XSCALE = 4.0          # fp8 pre-scale on x and W_ih
GSCALE = XSCALE * XSCALE  # resulting gate scale (W_hh also x16)

F32 = mybir.dt.float32
F16 = mybir.dt.float16
BF16 = mybir.dt.bfloat16
FP8 = mybir.dt.float8e4
I32 = mybir.dt.int32

AFT = mybir.ActivationFunctionType
DR = mybir.MatmulPerfMode.DoubleRow

# vocab-projection N-groups: (offset, size) within VL
PROJ_CHUNKS = [(0, 512), (512, 512), (1024, 226)]

# gate-type order: PyTorch i,f,g,o at gate-dim offsets 0,512,1024,1536.
# Emission order puts f first so sigmoid(f) can start the c-chain earliest.
TYPE_ORDER = (1, 0, 2, 3)


def build_nc(with_gate_bias: bool, with_fc_bias: bool):
    nc = bacc.Bacc("TRN2", target_bir_lowering=False, debug=False, num_devices=NCORES)

    xsT_d = nc.dram_tensor("xsTd", [128, KE * NTOK], FP8, kind="ExternalInput")
    g0_d = nc.dram_tensor("g0", [128, 16 * B], BF16, kind="ExternalInput")
    wxT_d = nc.dram_tensor("wxT", [2, 128, 2 * G4], FP8, kind="ExternalInput")
    whT_d = nc.dram_tensor("whT", [KH, 128, G4], BF16, kind="ExternalInput")
    fcT_d = nc.dram_tensor("fcT", [KH, 128, VL], BF16, kind="ExternalInput")
    bg_d = nc.dram_tensor("bg", [1, G4], BF16, kind="ExternalInput")
    fcb_d = nc.dram_tensor("fcb", [1, VL], BF16, kind="ExternalInput")
    out_d = nc.dram_tensor("out", [NTOK, VL], F16, kind="ExternalOutput")

    with tile.TileContext(nc) as tc:
        build_body(
            nc, tc,
            xsT_d, g0_d, wxT_d, whT_d, fcT_d, bg_d, fcb_d, out_d,
            with_gate_bias, with_fc_bias,
        )
    nc.compile()
    return nc


def build_body(nc, tc, xsT_d, g0_d, wxT_d, whT_d, fcT_d, bg_d, fcb_d,
               out_d, with_gate_bias, with_fc_bias):
    from contextlib import ExitStack

    ctx = ExitStack()
    with ctx:
        const = ctx.enter_context(tc.tile_pool(name="const", bufs=1))
        gatesp = ctx.enter_context(tc.tile_pool(name="gates", bufs=1, space="PSUM"))
        projp = ctx.enter_context(tc.tile_pool(name="projp", bufs=2, space="PSUM"))
        ew = ctx.enter_context(tc.tile_pool(name="ew", bufs=2))
        state = ctx.enter_context(tc.tile_pool(name="state", bufs=1))
        stage = ctx.enter_context(tc.tile_pool(name="stage", bufs=2))

        wx_sb = const.tile([128, 2 * 2 * G4], FP8)   # col = kp*4096 + s*2048 + g
        wh_sb = const.tile([128, KH * G4], BF16)     # col = k*2048 + g (x16)
        fc_sb = const.tile([128, KH * VL], BF16)     # col = k*VL + v
        g0_sb = const.tile([128, 16 * B], BF16)      # host-computed t=0 gates

        if with_gate_bias:
            bg_sb = const.tile([1, G4], BF16)        # (b_ih+b_hh) x16
        if with_fc_bias:
            fcb_sb = const.tile([1, VL], BF16)
        if with_gate_bias or with_fc_bias:
            ones_row = const.tile([1, 128], BF16)
            nc.gpsimd.memset(ones_row[:], 1.0)

        # xsT: xs.T fp8 (x4), E-chunk k at free offset k*NTOK
        xsT = const.tile([128, KE * NTOK], FP8)
        # h_seqT: h.T bf16 (true scale), H-chunk k at free offset k*NTOK
        h_seqT = const.tile([128, KH * NTOK], BF16)
        # c state bf16: keeps every c-chain DVE op all-2-byte (2x mode);
        # the forget-gate decay absorbs the per-step rounding (4.2e-3 model)
        c_t = state.tile([128, 256], BF16)

        gate_tiles = {}  # pair -> dict tau -> psum tile [128, 512]
        wx_v = wx_sb[:].rearrange("p (kp s g) -> p kp s g", kp=2, s=2)
        xs_v = xsT[:].rearrange("p (kp s t) -> p kp s t", kp=2, s=2)

        def emit_xproj(p):
            """fp8 DoubleRow x-part for pair p. Pair 0 only fills the odd
            (t=1) token half; the t=0 gates come from g0_sb."""
            off = B if p == 0 else 0
            tiles = {
                tau: gatesp.tile([128, 512], F32, name=f"g{tau}", tag=f"g{tau}")
                for tau in TYPE_ORDER
            }
            # kp-outer so pass 0 only needs the first wx half (preamble).
            # PSUM accumulation: start=True clears the has-written bits of
            # the WHOLE bank, so it may appear exactly once per bank per
            # generation, on the bank's first write; later start=False
            # writes overwrite untouched regions and accumulate touched
            # ones (see trainium-docs memories/02-psum.md).
            for kp in range(2):
                for tau in TYPE_ORDER:
                    g = tiles[tau]
                    for c in range(4):
                        nc.tensor.matmul(
                            g[:, c * 128 + off:(c + 1) * 128],
                            lhsT=wx_v[:, kp, :, tau * 512 + c * 128:
                                      tau * 512 + (c + 1) * 128],
                            rhs=xs_v[:, kp, :, p * 128 + off:(p + 1) * 128],
                            start=(kp == 0 and c == 0), stop=False,
                            perf_mode=DR, skip_group_check=True,
                        )
            if with_gate_bias:
                for tau in TYPE_ORDER:
                    g = tiles[tau]
                    for c in range(4):
                        nc.tensor.matmul(
                            g[:, c * 128 + off:(c + 1) * 128],
                            lhsT=bg_sb[:1, tau * 512 + c * 128:
                                       tau * 512 + (c + 1) * 128],
                            rhs=ones_row[:1, off:128],
                            start=False, stop=False, skip_group_check=True,
                        )
            gate_tiles[p] = tiles

        def emit_hpart(t):
            """h-part for step t: accumulate (16*W_hh.T) @ h_{t-1}.T into the
            token-half ht of pair p's gate banks. k-mid order so the first
            half of h_{t-1} suffices to start each type block."""
            p, ht = divmod(t, 2)
            tiles = gate_tiles[p]
            for tau in TYPE_ORDER:
                g = tiles[tau]
                for k in range(KH):
                    for c in range(4):
                        col = c * 128 + ht * 64
                        nc.tensor.matmul(
                            g[:, col:col + 64],
                            lhsT=wh_sb[:, k * G4 + tau * 512 + c * 128:
                                       k * G4 + tau * 512 + (c + 1) * 128],
                            rhs=h_seqT[:, k * NTOK + (t - 1) * B:
                                       k * NTOK + t * B],
                            start=False, stop=(k == KH - 1 and c == 3),
                            skip_group_check=True,
                        )

        def gate_ap(g, ht):
            return g[:].rearrange("p (c t) -> p c t", c=4)[:, :, ht * 64:(ht + 1) * 64]

        g0_v = g0_sb[:].rearrange("p (cc t) -> p cc t", cc=16)

        def emit_elementwise(t):
            p, ht = divmod(t, 2)
            if t == 0:
                ins = {tau: g0_v[:, tau * 4:(tau + 1) * 4, :] for tau in range(4)}
                sc = 1.0
            else:
                tiles = gate_tiles[p]
                ins = {tau: gate_ap(tiles[tau], ht) for tau in range(4)}
                sc = 1.0 / GSCALE
            f_s = ew.tile([128, 256], BF16, name="f_s", tag="f_s")
            nc.scalar.activation(f_s[:], ins[1], AFT.Sigmoid, scale=sc)
            i_s = ew.tile([128, 256], BF16, name="i_s", tag="i_s")
            nc.scalar.activation(i_s[:], ins[0], AFT.Sigmoid, scale=sc)
            g_t = ew.tile([128, 256], BF16, name="g_t", tag="g_t")
            nc.scalar.activation(g_t[:], ins[2], AFT.Tanh, scale=sc)
            o_s = ew.tile([128, 256], BF16, name="o_s", tag="o_s")
            nc.scalar.activation(o_s[:], ins[3], AFT.Sigmoid, scale=sc)

            if t == 0:
                nc.vector.tensor_mul(c_t[:], i_s[:], g_t[:])
            else:
                fc_ = ew.tile([128, 256], BF16, name="fc_", tag="fc_")
                # u in bf16: all-2-byte operands get the DVE 2x perf mode
                u = ew.tile([128, 256], BF16, name="u", tag="u")
                # halves so tanh(c) half 0 can start before half 1's update
                for hf in range(2):
                    sl = slice(hf * 128, (hf + 1) * 128)
                    nc.vector.tensor_mul(fc_[:, sl], f_s[:, sl], c_t[:, sl])
                    nc.vector.tensor_mul(u[:, sl], i_s[:, sl], g_t[:, sl])
                    nc.vector.tensor_add(c_t[:, sl], fc_[:, sl], u[:, sl])
            tc_t = ew.tile([128, 256], BF16, name="tc_t", tag="tc_t")
            hv = h_seqT[:].rearrange("p (k t) -> p k t", k=KH)
            ov = o_s[:].rearrange("p (k t) -> p k t", k=2, t=128)
            cv = tc_t[:].rearrange("p (k t) -> p k t", k=2, t=128)
            # halves: tanh(c) then h per half so the next step's k01 h-part
            # matmuls can start before the second half lands.
            for hf in range(2):
                nc.scalar.activation(tc_t[:, hf * 128:(hf + 1) * 128],
                                     c_t[:, hf * 128:(hf + 1) * 128], AFT.Tanh)
                nc.vector.tensor_mul(
                    hv[:, 2 * hf:2 * hf + 2, t * B:(t + 1) * B],
                    ov[:, hf, :].rearrange("p (k t) -> p k t", k=2),
                    cv[:, hf, :].rearrange("p (k t) -> p k t", k=2),
                )

        drain_engines = (nc.vector.tensor_copy, nc.scalar.copy,
                         nc.vector.tensor_copy)
        proj_stage = {}

        def emit_proj_groups(m, groups):
            """Vocab projection groups for token chunk m; fp16 staging, one
            output DMA once group 2 lands."""
            if m not in proj_stage:
                proj_stage[m] = stage.tile([128, VL], F16, name="st", tag="st")
            st = proj_stage[m]
            for j in groups:
                n0, nsz = PROJ_CHUNKS[j]
                pj = projp.tile([128, 512], F32, name="pj", tag="pj")
                for k in range(KH):
                    nc.tensor.matmul(
                        pj[:, :nsz],
                        lhsT=h_seqT[:, k * NTOK + m * 128:k * NTOK + (m + 1) * 128],
                        rhs=fc_sb[:, k * VL + n0:k * VL + n0 + nsz],
                        start=(k == 0),
                        stop=(k == KH - 1) and not with_fc_bias,
                        skip_group_check=True,
                    )
                if with_fc_bias:
                    nc.tensor.matmul(
                        pj[:, :nsz],
                        lhsT=ones_row[:1, :],
                        rhs=fcb_sb[:1, n0:n0 + nsz],
                        start=False, stop=True, skip_group_check=True,
                    )
                drain_engines[j](st[:, n0:n0 + nsz], pj[:, :nsz])
                if j == len(PROJ_CHUNKS) - 1:
                    nc.sync.dma_start(out_d[m * 128:(m + 1) * 128, :], st[:])
                    del proj_stage[m]

        def emit_proj_chunk(m):
            emit_proj_groups(m, (0, 1, 2))

        # ---- schedule ----
        # embeddings for the first pairs before the (big) weight DMAs so the
        # PE can start transposing as soon as the gathers land; weight DMAs
        # on the SP queue ordered by first use (wx -> wh -> fc).
        xsT_v = xsT_d.ap().rearrange("p (k t) -> p k t", k=KE)
        xsb_v = xsT[:].rearrange("p (k t) -> p k t", k=KE)
        nc.scalar.dma_start(xsb_v[:, :, 0:512], xsT_v[:, :, 0:512])
        nc.scalar.dma_start(g0_sb[:], g0_d.ap())
        for kp in range(2):
            nc.sync.dma_start(wx_sb[:, kp * 4096:(kp + 1) * 4096], wxT_d[kp])
        for k in range(KH):
            nc.sync.dma_start(wh_sb[:, k * G4:(k + 1) * G4], whT_d[k])
        for k in range(KH):
            nc.sync.dma_start(fc_sb[:, k * VL:(k + 1) * VL], fcT_d[k])
        if with_gate_bias:
            nc.sync.dma_start(bg_sb[:], bg_d.ap())
        if with_fc_bias:
            nc.sync.dma_start(fcb_sb[:], fcb_d.ap())
        for q in range(1, 4):
            nc.sync.dma_start(xsb_v[:, :, q * 512:(q + 1) * 512],
                              xsT_v[:, :, q * 512:(q + 1) * 512])
        emit_xproj(0)
        # proj groups are PE fill between recurrence steps: 2 groups after
        # even steps, 1 after odd steps (before x_proj, covering its wait on
        # the previous pair's gate reads).
        for t in range(T):
            p, ht = divmod(t, 2)
            if t > 0:
                emit_hpart(t)
            # elementwise before the proj groups: the in-order ACT/DVE
            # queues must run this step's gate reads ahead of proj drains.
            emit_elementwise(t)
            if ht == 0 and p >= 1:
                emit_proj_groups(p - 1, (0, 1))
            # x_proj(p+1) reuses pair p's PSUM banks (bufs=1): it must be
            # emitted after this step's gate reads so the pool sees the WAR.
            if ht == 1:
                if p >= 1:
                    emit_proj_groups(p - 1, (2,))
                if p + 1 < NCHUNK:
                    emit_xproj(p + 1)
        emit_proj_chunk(NCHUNK - 1)


_CACHE = {}


def _get_nc(with_gate_bias, with_fc_bias):
    key = (with_gate_bias, with_fc_bias)
    if key not in _CACHE:
        _CACHE[key] = build_nc(with_gate_bias, with_fc_bias)
    return _CACHE[key]


LAST_RESULTS = None


def kernel(features, captions, embed_W, W_ih, W_hh, b_ih, b_hh, fc_W, fc_b,
           _trace=False):
    global LAST_RESULTS
    features = np.asarray(features, dtype=np.float32)
    captions = np.asarray(captions)
    embed_W = np.asarray(embed_W, dtype=np.float32)
    W_ih = np.asarray(W_ih, dtype=np.float32)
    W_hh = np.asarray(W_hh, dtype=np.float32)
    b_ih = np.asarray(b_ih, dtype=np.float32)
    b_hh = np.asarray(b_hh, dtype=np.float32)
    fc_W = np.asarray(fc_W, dtype=np.float32)
    fc_b = np.asarray(fc_b, dtype=np.float32)

    with_gate_bias = bool(np.any(b_ih) or np.any(b_hh))
    with_fc_bias = bool(np.any(fc_b))
    nc = _get_nc(with_gate_bias, with_fc_bias)

    bf16 = ml_dtypes.bfloat16
    fp8 = ml_dtypes.float8_e4m3

    # host-side embedding gather into the transposed fp8 xsT layout
    # (xsT[p, k*NTOK + tok] = fp8(bf16(4*emb))[token tok, e-dim k*128+p];
    # t=0 token slots are unused zeros - step 0 reads the host gates g0).
    idx = np.zeros((T, B), np.int64)
    idx[1:] = captions.astype(np.int64).T[1:]
    embQ = (embed_W * XSCALE).astype(bf16)
    xq = embQ[idx.reshape(NTOK)].astype(fp8)      # [NTOK, E]
    xq[0:B] = 0
    xsTh = np.ascontiguousarray(xq.T.reshape(KE, 128, NTOK)
                                .transpose(1, 0, 2).reshape(128, KE * NTOK))
    # wxT[kp, p, s*2048+g] = (W_ih.T * XSCALE)[(2kp+s)*128+p, g]
    wxT = np.ascontiguousarray(
        (W_ih.T * XSCALE).astype(fp8).reshape(2, 2, 128, G4).transpose(0, 2, 1, 3)
        .reshape(2, 128, 2 * G4))
    whT = np.ascontiguousarray(W_hh.T * GSCALE).astype(bf16).reshape(KH, 128, G4)
    fcT_full = np.ascontiguousarray(fc_W.T).astype(bf16)  # [H, V]
    bg = ((b_ih + b_hh) * GSCALE).astype(bf16).reshape(1, G4)

    # host-computed t=0 gates (f32): g0_sb[p, cc*64+b] = g0[b, cc*128+p]
    g0 = features @ W_ih.T + b_ih + b_hh          # [B, G4]
    g0T = np.ascontiguousarray(g0.T)              # [G4, B]
    g0u = np.ascontiguousarray(
        g0T.reshape(16, 128, B).transpose(1, 0, 2).reshape(128, 16 * B)
    ).astype(bf16)

    in_maps = []
    for c in range(NCORES):
        in_maps.append({
            "xsTd": xsTh,
            "g0": g0u,
            "wxT": wxT,
            "whT": whT,
            "fcT": np.ascontiguousarray(
                fcT_full[:, c * VL:(c + 1) * VL].reshape(KH, 128, VL)),
            "bg": bg,
            "fcb": fc_b[c * VL:(c + 1) * VL].astype(bf16).reshape(1, VL),
        })

    try:
        res = run_bass_kernel_spmd(nc, in_maps, list(range(NCORES)), trace=_trace)
    except ModuleNotFoundError:
        res = run_bass_kernel_spmd(nc, in_maps, list(range(NCORES)))
    LAST_RESULTS = res

    outs = [
        res.results[c]["out"].astype(np.float32).reshape(T, B, VL).transpose(1, 0, 2)
        for c in range(NCORES)
    ]
    return np.ascontiguousarray(np.concatenate(outs, axis=2), dtype=np.float32)
